# revision 6
# baseline (speedup 1.0000x reference)
"""Trainium2 Bass kernel for nn_Attention_87625922773715.

Self-attention block (SAGAN-style) on [8, 256, 64, 64]:
  theta = IN(conv1x1(x, theta_w));  phi = IN(maxpool2(conv1x1(x, phi_w)))
  g     = IN(maxpool2(conv1x1(x, g_w)))
  beta  = softmax(theta_h^T phi_h) per head (head dim inner, 4 heads)
  out   = gamma * conv1x1(attn(g, beta), o_w) + x

Sharding: data-parallel over batch B=8 across the 8 NeuronCores (one
sample per core); all weights replicated.

Key device-side design decisions:
  * Channel dims are permuted to head-major order on the host (weights /
    norm affines permuted to match) so per-head slices are contiguous
    partition ranges.
  * Spatial columns are permuted on the host ("pi" order) so each 2x2
    maxpool window is 4 consecutive columns -> pool is a single
    reduce_max over a [128, n, 4] view of the conv PSUM tile.
  * Attention is computed key-major: S^T[j, i] = phi_h^T theta_h so the
    softmax exp output P^T is directly the K-operand of the o-matmul.
    exp() skips max-subtraction (|logits| < 60, safe in fp32).
  * The softmax denominator comes free from an appended ones-column in
    the o-matmul weights (output row 32); o is divided by it after the
    o accumulation, via a K=4 one-hot broadcast matmul.
  * g's instance-norm is algebraically folded: static affine (ng_w,
    ng_b, gamma) into o_w on the host; dynamic standardization
    (rstd, -mean*rstd) into o_w on the device (uses sum_j beta = 1).
  * rsqrt is computed as exp(-0.5*ln(v+eps)) to stay inside the
    natural_log_exp activation table set (no table switches).
"""

import os
import sys

sys.path.insert(0, "/opt/trn_rl_repo")

import numpy as np

import concourse.bacc as bacc
import concourse.tile as tile
from concourse import mybir
from concourse.bass_utils import run_bass_kernel_spmd
from concourse.masks import make_identity

F32 = mybir.dt.float32
Alu = None  # set lazily below to mybir.AluOpType
Act = None  # mybir.ActivationFunctionType

B, C, H, W, HEADS = 8, 256, 64, 64, 4
N = H * W          # 4096 queries
M = N // 4         # 1024 keys (after 2x2 pool)
CH = C // HEADS    # 64 channels per head (theta/phi)
CG = (C // 2) // HEADS  # 32 channels per head (g)
EPS = 1e-5
NB = 128           # partition block

JT = N // 512      # 8 column tiles of 512
NCHUNK = 1024      # attention i-chunk width
NCH = N // NCHUNK  # 4 chunks
JB = M // 128      # 8 key blocks of 128


def _perm_headmajor(channels, heads):
    """perm[r] maps head-major row r = h*per_head + cc -> original channel
    cc*heads + h."""
    per_head = channels // heads
    r = np.arange(channels)
    cc, h = r % per_head, r // per_head
    return (cc * heads + h).astype(np.int64)


def _pi_index():
    """pi[k] = original spatial column for permuted position k; each
    consecutive 4-run is one 2x2 maxpool window."""
    idx = np.empty(N, np.int64)
    for p in range(M):
        hp, wp = p // (W // 2), p % (W // 2)
        for d in range(4):
            dh, dw = d // 2, d % 2
            idx[p * 4 + d] = (2 * hp + dh) * W + (2 * wp + dw)
    return idx


_PERM_T = _perm_headmajor(C, HEADS)
_PERM_G = _perm_headmajor(C // 2, HEADS)
_PI = _pi_index()


def build_nc():
    global Alu, Act
    Alu = mybir.AluOpType
    Act = mybir.ActivationFunctionType

    nc = bacc.Bacc("TRN2", target_bir_lowering=False)

    x_d = nc.dram_tensor("x", [C, N], F32, kind="ExternalInput")
    tw_d = nc.dram_tensor("tw", [C, C], F32, kind="ExternalInput")   # theta lhsT
    pw_d = nc.dram_tensor("pw", [C, C], F32, kind="ExternalInput")   # phi lhsT
    gw_d = nc.dram_tensor("gw", [C, C // 2], F32, kind="ExternalInput")
    ow_d = nc.dram_tensor("ow", [C // 2, C], F32, kind="ExternalInput")  # ow2T
    # prm columns: 0 nt_w, 1 nt_b, 2 np_w, 3 np_b, 4 c0 (c0 by original chan)
    prm_d = nc.dram_tensor("prm", [C, 5], F32, kind="ExternalInput")
    out_d = nc.dram_tensor("out", [C, N], F32, kind="ExternalOutput")

    with tile.TileContext(nc) as tc:
        with tc.tile_pool(name="persist", bufs=1) as pp:
            # ---- persistent tiles -------------------------------------
            x_sb = [pp.tile([NB, N], F32, name=f"x{m}", tag=f"x{m}") for m in range(2)]
            tw_sb = [pp.tile([NB, C], F32, name=f"tw{k}", tag=f"tw{k}") for k in range(2)]
            pw_sb = [pp.tile([NB, C], F32, name=f"pw{k}", tag=f"pw{k}") for k in range(2)]
            gw_sb = [pp.tile([NB, C // 2], F32, name=f"gw{k}", tag=f"gw{k}") for k in range(2)]
            ow_sb = pp.tile([NB, C], F32, name="ow", tag="ow")
            ow3_sb = pp.tile([NB, C], F32, name="ow3", tag="ow3")
            prm_sb = [pp.tile([NB, 5], F32, name=f"prm{m}", tag=f"prm{m}") for m in range(2)]
            ident = pp.tile([NB, NB], F32, name="ident", tag="ident")
            theta_sb = [pp.tile([NB, N], F32, name=f"theta{m}", tag=f"theta{m}") for m in range(2)]
            phi_sb = [pp.tile([NB, M], F32, name=f"phi{m}", tag=f"phi{m}") for m in range(2)]
            g01_sb = pp.tile([2 * CG, M], F32, name="g01", tag="g01")
            g23_sb = pp.tile([2 * CG, M], F32, name="g23", tag="g23")
            gt_sb = [pp.tile([NB, JB, CG + 1], F32, name=f"gt{h}", tag=f"gt{h}")
                     for h in range(HEADS)]
            o_sb = pp.tile([NB, N], F32, name="ostage", tag="ostage")
            rec_sb = pp.tile([NB, N], F32, name="rec", tag="rec")
            ones_c = pp.tile([NB, CG], F32, name="ones_c", tag="ones_c")
            eps_sb = pp.tile([NB, 1], F32, name="eps", tag="eps")
            cbias = [pp.tile([NB, 1], F32, name=f"cbias{m}", tag=f"cbias{m}") for m in range(2)]

            nc.sync.dma_start(out=x_sb[0], in_=x_d[0:NB, :])
            nc.sync.dma_start(out=x_sb[1], in_=x_d[NB:C, :])
            for k in range(2):
                nc.sync.dma_start(out=tw_sb[k], in_=tw_d[k * NB:(k + 1) * NB, :])
                nc.sync.dma_start(out=pw_sb[k], in_=pw_d[k * NB:(k + 1) * NB, :])
                nc.sync.dma_start(out=gw_sb[k], in_=gw_d[k * NB:(k + 1) * NB, :])
            nc.sync.dma_start(out=ow_sb, in_=ow_d[:, :])
            for m in range(2):
                nc.sync.dma_start(out=prm_sb[m], in_=prm_d[m * NB:(m + 1) * NB, :])
            make_identity(nc, ident)
            nc.vector.memset(eps_sb, EPS)
            nc.gpsimd.memset(ones_c, 1.0)
            for h in range(HEADS):
                nc.gpsimd.memset(gt_sb[h], 1.0)  # ones column survives at [:, :, CG]

            def rstd_from_var(var_ap, out_ap, parts):
                """out = 1/sqrt(var + EPS) via exp(-0.5 * ln(var + eps))."""
                nc.scalar.activation(out=out_ap, in_=var_ap, func=Act.Ln,
                                     bias=eps_sb[0:parts, :], scale=1.0)
                nc.scalar.activation(out=out_ap, in_=out_ap, func=Act.Exp,
                                     bias=0.0, scale=-0.5)

            # ---- stage 1: theta / phi / g convs + instance norms ------
            with tc.tile_pool(name="convps", bufs=4, space="PSUM") as cps, \
                 tc.tile_pool(name="stats", bufs=1) as stp:
                # theta
                t_stats = [stp.tile([NB, JT, 6], F32, name=f"ts{m}", tag=f"ts{m}") for m in range(2)]
                for m in range(2):
                    for jt in range(JT):
                        ps = cps.tile([NB, 512], F32, name="conv", tag="conv")
                        js = slice(jt * 512, (jt + 1) * 512)
                        nc.tensor.matmul(ps, lhsT=tw_sb[0][:, m * NB:(m + 1) * NB],
                                         rhs=x_sb[0][:, js], start=True, stop=False)
                        nc.tensor.matmul(ps, lhsT=tw_sb[1][:, m * NB:(m + 1) * NB],
                                         rhs=x_sb[1][:, js], start=False, stop=True)
                        nc.vector.bn_stats(out=t_stats[m][:, jt, :], in_=ps)
                        nc.vector.tensor_copy(out=theta_sb[m][:, js], in_=ps)
                # phi (pooled)
                p_stats = [stp.tile([NB, 2, 6], F32, name=f"ps{m}", tag=f"ps{m}") for m in range(2)]
                for m in range(2):
                    for jt in range(JT):
                        ps = cps.tile([NB, 512], F32, name="conv", tag="conv")
                        js = slice(jt * 512, (jt + 1) * 512)
                        nc.tensor.matmul(ps, lhsT=pw_sb[0][:, m * NB:(m + 1) * NB],
                                         rhs=x_sb[0][:, js], start=True, stop=False)
                        nc.tensor.matmul(ps, lhsT=pw_sb[1][:, m * NB:(m + 1) * NB],
                                         rhs=x_sb[1][:, js], start=False, stop=True)
                        nc.vector.reduce_max(
                            out=phi_sb[m][:, jt * 128:(jt + 1) * 128],
                            in_=ps.rearrange("p (a b) -> p a b", b=4),
                            axis=mybir.AxisListType.X)
                # g (pooled, split into two 64-partition tiles)
                g_stats = [stp.tile([2 * CG, 2, 6], F32, name=f"gs{m}", tag=f"gs{m}") for m in range(2)]
                for jt in range(JT):
                    ps = cps.tile([NB, 512], F32, name="conv", tag="conv")
                    js = slice(jt * 512, (jt + 1) * 512)
                    nc.tensor.matmul(ps, lhsT=gw_sb[0], rhs=x_sb[0][:, js],
                                     start=True, stop=False)
                    nc.tensor.matmul(ps, lhsT=gw_sb[1], rhs=x_sb[1][:, js],
                                     start=False, stop=True)
                    for half, gdst in ((0, g01_sb), (1, g23_sb)):
                        nc.vector.reduce_max(
                            out=gdst[:, jt * 128:(jt + 1) * 128],
                            in_=ps[half * 64:(half + 1) * 64, :].rearrange(
                                "p (a b) -> p a b", b=4),
                            axis=mybir.AxisListType.X)

                # theta norm finalize + affine
                for m in range(2):
                    mv = stp.tile([NB, 2], F32, name=f"tmv{m}", tag=f"tmv{m}")
                    nc.vector.bn_aggr(out=mv, in_=t_stats[m])
                    rstd = stp.tile([NB, 1], F32, name=f"trs{m}", tag=f"trs{m}")
                    rstd_from_var(mv[:, 1:2], rstd, NB)
                    scale = stp.tile([NB, 1], F32, name=f"tsc{m}", tag=f"tsc{m}")
                    nc.vector.tensor_mul(out=scale, in0=rstd, in1=prm_sb[m][:, 0:1])
                    bias = stp.tile([NB, 1], F32, name=f"tbi{m}", tag=f"tbi{m}")
                    nc.vector.tensor_mul(out=bias, in0=mv[:, 0:1], in1=scale)
                    nc.vector.tensor_sub(out=bias, in0=prm_sb[m][:, 1:2], in1=bias)
                    nc.vector.tensor_scalar(out=theta_sb[m], in0=theta_sb[m],
                                            scalar1=scale, scalar2=bias,
                                            op0=Alu.mult, op1=Alu.add)
                # phi norm finalize + affine
                for m in range(2):
                    for s in range(2):
                        nc.vector.bn_stats(out=p_stats[m][:, s, :],
                                           in_=phi_sb[m][:, s * 512:(s + 1) * 512])
                    mv = stp.tile([NB, 2], F32, name=f"pmv{m}", tag=f"pmv{m}")
                    nc.vector.bn_aggr(out=mv, in_=p_stats[m])
                    rstd = stp.tile([NB, 1], F32, name=f"prs{m}", tag=f"prs{m}")
                    rstd_from_var(mv[:, 1:2], rstd, NB)
                    scale = stp.tile([NB, 1], F32, name=f"psc{m}", tag=f"psc{m}")
                    nc.vector.tensor_mul(out=scale, in0=rstd, in1=prm_sb[m][:, 2:3])
                    bias = stp.tile([NB, 1], F32, name=f"pbi{m}", tag=f"pbi{m}")
                    nc.vector.tensor_mul(out=bias, in0=mv[:, 0:1], in1=scale)
                    nc.vector.tensor_sub(out=bias, in0=prm_sb[m][:, 3:4], in1=bias)
                    nc.vector.tensor_scalar(out=phi_sb[m], in0=phi_sb[m],
                                            scalar1=scale, scalar2=bias,
                                            op0=Alu.mult, op1=Alu.add)
                # g norm -> fold into ow3 = ow2T * rstd ; beta = -mean*rstd
                alpha = stp.tile([NB, 1], F32, name="galpha", tag="galpha")
                beta = stp.tile([NB, 1], F32, name="gbeta", tag="gbeta")
                for half, gsrc in ((0, g01_sb), (1, g23_sb)):
                    for s in range(2):
                        nc.vector.bn_stats(out=g_stats[half][:, s, :],
                                           in_=gsrc[:, s * 512:(s + 1) * 512])
                    mv = stp.tile([2 * CG, 2], F32, name=f"gmv{half}", tag=f"gmv{half}")
                    nc.vector.bn_aggr(out=mv, in_=g_stats[half])
                    pa = slice(half * 64, (half + 1) * 64)
                    rstd = stp.tile([2 * CG, 1], F32, name=f"grs{half}", tag=f"grs{half}")
                    rstd_from_var(mv[:, 1:2], rstd, 2 * CG)
                    nc.vector.tensor_copy(out=alpha[pa, :], in_=rstd)
                    nb_t = stp.tile([2 * CG, 1], F32, name=f"gnb{half}", tag=f"gnb{half}")
                    nc.vector.tensor_mul(out=nb_t, in0=mv[:, 0:1], in1=rstd)
                    nc.vector.tensor_scalar_mul(out=beta[pa, :], in0=nb_t,
                                                scalar1=-1.0)
                nc.vector.tensor_scalar_mul(out=ow3_sb, in0=ow_sb, scalar1=alpha)
                for m in range(2):
                    psc = cps.tile([NB, 1], F32, name="c1", tag="c1")
                    nc.tensor.matmul(psc, lhsT=ow_sb[:, m * NB:(m + 1) * NB],
                                     rhs=beta, start=True, stop=True)
                    nc.vector.tensor_add(out=cbias[m], in0=psc,
                                         in1=prm_sb[m][:, 4:5])

            # ---- stage 2: transpose g (raw) into o-matmul weights -----
            with tc.tile_pool(name="trps", bufs=2, space="PSUM") as tps:
                for h in range(HEADS):
                    gsrc = g01_sb if h < 2 else g23_sb
                    b0 = (h % 2) * CG
                    for jb in range(JB):
                        pst = tps.tile([NB, CG], F32, name="tr", tag="tr")
                        nc.tensor.transpose(
                            pst, gsrc[b0:b0 + CG, jb * 128:(jb + 1) * 128],
                            ident[b0:b0 + CG, b0:b0 + CG])
                        nc.vector.tensor_copy(out=gt_sb[h][:, jb, 0:CG], in_=pst)

            # ---- stage 3: attention ----------------------------------
            with tc.tile_pool(name="stps", bufs=2, space="PSUM") as stps, \
                 tc.tile_pool(name="ops", bufs=4, space="PSUM") as ops, \
                 tc.tile_pool(name="ptp", bufs=9) as ptp:
                for h in range(HEADS):
                    tsrc = theta_sb[h // 2]
                    psrc = phi_sb[h // 2]
                    rows = slice((h % 2) * CH, (h % 2 + 1) * CH)
                    for c in range(NCH):
                        ps_o = [ops.tile([CG + 1, 512], F32, name="pso", tag="pso")
                                for _ in range(2)]
                        for jb in range(JB):
                            ps_st = stps.tile([NB, NCHUNK], F32, name="st", tag="st")
                            for it in range(2):
                                i0 = c * NCHUNK + it * 512
                                nc.tensor.matmul(
                                    ps_st[:, it * 512:(it + 1) * 512],
                                    lhsT=psrc[rows, jb * 128:(jb + 1) * 128],
                                    rhs=tsrc[rows, i0:i0 + 512],
                                    start=True, stop=True)
                            pt = ptp.tile([NB, NCHUNK], F32, name="pt", tag="pt")
                            nc.scalar.activation(out=pt, in_=ps_st, func=Act.Exp)
                            for it in range(2):
                                nc.tensor.matmul(
                                    ps_o[it], lhsT=gt_sb[h][:, jb, :],
                                    rhs=pt[:, it * 512:(it + 1) * 512],
                                    start=(jb == 0), stop=(jb == JB - 1))
                        for it in range(2):
                            i0 = c * NCHUNK + it * 512
                            nc.vector.tensor_copy(
                                out=o_sb[h * CG:(h + 1) * CG, i0:i0 + 512],
                                in_=ps_o[it][0:CG, :])
                            nc.vector.reciprocal(
                                out=rec_sb[h * CG:h * CG + 1, i0:i0 + 512],
                                in_=ps_o[it][CG:CG + 1, :])

            # ---- stage 4: normalize o by softmax denominator ----------
            with tc.tile_pool(name="rbps", bufs=2, space="PSUM") as rbps, \
                 tc.tile_pool(name="ocps", bufs=2, space="PSUM") as ocps:
                for jt in range(JT):
                    js = slice(jt * 512, (jt + 1) * 512)
                    psr = rbps.tile([NB, 512], F32, name="rb", tag="rb")
                    for h in range(HEADS):
                        nc.tensor.matmul(psr[h * CG:(h + 1) * CG, :],
                                         lhsT=ones_c[h * CG:h * CG + 1, :],
                                         rhs=rec_sb[h * CG:h * CG + 1, js],
                                         start=True, stop=True,
                                         tile_position=(h * CG, h * CG))
                    nc.vector.tensor_mul(out=o_sb[:, js], in0=o_sb[:, js], in1=psr)

                # ---- stage 5: output conv + residual ------------------
                for m in range(2):
                    for jt in range(JT):
                        js = slice(jt * 512, (jt + 1) * 512)
                        pso = ocps.tile([NB, 512], F32, name="oc", tag="oc")
                        nc.tensor.matmul(pso, lhsT=ow3_sb[:, m * NB:(m + 1) * NB],
                                         rhs=o_sb[:, js], start=True, stop=True)
                        # out = (oc + cbias) + x ; theta tile is dead - reuse
                        res = theta_sb[m]
                        nc.vector.scalar_tensor_tensor(
                            out=res[:, js], in0=pso, scalar=cbias[m],
                            in1=x_sb[m][:, js], op0=Alu.add, op1=Alu.add)
                        nc.sync.dma_start(out=out_d[m * NB:(m + 1) * NB, js],
                                          in_=res[:, js])

    nc.compile()
    return nc


def prepare_core_inputs(inputs):
    """Host-side preprocessing -> list of 8 per-core input dicts."""
    x = np.asarray(inputs["x"], np.float32)
    theta_w = np.asarray(inputs["theta_w"], np.float32)
    phi_w = np.asarray(inputs["phi_w"], np.float32)
    g_w = np.asarray(inputs["g_w"], np.float32)
    o_w = np.asarray(inputs["o_w"], np.float32)
    nt_w = np.asarray(inputs["nt_w"], np.float32)
    nt_b = np.asarray(inputs["nt_b"], np.float32)
    np_w = np.asarray(inputs["np_w"], np.float32)
    np_b = np.asarray(inputs["np_b"], np.float32)
    ng_w = np.asarray(inputs["ng_w"], np.float32)
    ng_b = np.asarray(inputs["ng_b"], np.float32)
    gamma = float(np.asarray(inputs["gamma"]))

    xf = x.reshape(B, C, N)[:, :, _PI]               # pi-permuted columns
    tw = np.ascontiguousarray(theta_w[_PERM_T, :].T)  # [C(k), C(m head-major)]
    pw = np.ascontiguousarray(phi_w[_PERM_T, :].T)
    gw = np.ascontiguousarray(g_w[_PERM_G, :].T)      # [C(k), 128]
    ow_p = o_w[:, _PERM_G]                            # [C, 128] head-major cols
    ow2 = ow_p * (gamma * ng_w[_PERM_G])[None, :]
    owT = np.ascontiguousarray(ow2.T)                 # [128, C]
    c0 = gamma * (ow_p @ ng_b[_PERM_G])               # [C]
    prm = np.zeros((C, 5), np.float32)
    prm[:, 0] = nt_w[_PERM_T]
    prm[:, 1] = nt_b[_PERM_T]
    prm[:, 2] = np_w[_PERM_T]
    prm[:, 3] = np_b[_PERM_T]
    prm[:, 4] = c0
    shared = {
        "tw": tw, "pw": pw, "gw": gw, "ow": owT.astype(np.float32),
        "prm": prm,
    }
    return [dict(shared, x=np.ascontiguousarray(xf[b])) for b in range(B)]


def assemble_output(outs):
    """outs: list of 8 [C, N] arrays in pi order -> [B, C, H, W]."""
    full = np.stack(outs, 0)
    res = np.empty((B, C, N), np.float32)
    res[:, :, _PI] = full
    return res.reshape(B, C, H, W)


_NC_CACHE = []


def _get_nc():
    if not _NC_CACHE:
        _NC_CACHE.append(build_nc())
    return _NC_CACHE[0]


def run_on_hw(inputs, trace=False):
    nc = _get_nc()
    in_maps = prepare_core_inputs(inputs)
    res = run_bass_kernel_spmd(nc, in_maps, core_ids=list(range(B)), trace=trace)
    out = assemble_output([res.results[i]["out"] for i in range(B)])
    return out, res


def kernel(**inputs):
    out, _ = run_on_hw(inputs, trace=False)
    return out


# revision 25
# speedup vs baseline: 1.8472x; 1.8472x over previous
"""Trainium2 Bass kernel for nn_Attention_87625922773715.

Self-attention block (SAGAN-style) on [8, 256, 64, 64]:
  theta = IN(conv1x1(x, theta_w));  phi = IN(maxpool2(conv1x1(x, phi_w)))
  g     = IN(maxpool2(conv1x1(x, g_w)))
  beta  = softmax(theta_h^T phi_h) per head (head dim inner, 4 heads)
  out   = gamma * conv1x1(attn(g, beta), o_w) + x

Sharding: data-parallel over batch B=8 across the 8 NeuronCores (one
sample per core); all weights replicated.

Device-side design:
  * Channel dims permuted to head-major on the host (weights / norm
    affines permuted to match) so per-head slices are contiguous
    partition ranges.
  * Spatial columns permuted on the host ("pi" order) so each 2x2
    maxpool window is 4 consecutive columns -> maxpool is one
    reduce_max over a [128, n, 4] view of the conv PSUM tile.
  * Attention is computed key-major: S^T[j, i] = phi_h^T theta_h so the
    softmax exp output P^T is directly the K-operand of the o-matmul.
    exp() skips max-subtraction (|logits| < 60, safe: P <= e^57, sums
    < 6e24, all within fp32/bf16 exponent range).
  * Head pairs (partition bases 0/64 of the same tile) issue their K=64
    logit matmuls back-to-back so the PE runs them in disjoint
    row-groups concurrently.
  * The softmax denominator comes free from an appended ones-column in
    the o-matmul weights (psum row 32); its reciprocal is computed as
    exp(-ln(d*2^-40) - 40ln2) on ScalarE (DVE reciprocal is ~8cyc/elem
    and custom-DVE approx ops don't execute through this runtime), on
    the ones-matmul-broadcast [128,512] denominator.
  * g's instance norm is folded away: static affine (ng_w, ng_b, gamma)
    into o_w on the host; dynamic standardization (rstd, -mean*rstd)
    into o_w / a bias term on the device (uses sum_j softmax = 1).
  * All big matmuls run in bf16 (1 cycle/row on the PE vs 4 for fp32,
    ~2.6 measured for f32r); accumulation stays fp32 in PSUM, the
    residual path (x, +) stays fully fp32.
  * rsqrt for the instance norms = exp(-0.5*ln(v+eps)); all Ln ops are
    grouped before all Exp ops to minimize ACT table-set switches.
"""

import os
import sys

sys.path.insert(0, "/opt/trn_rl_repo")

import ml_dtypes
import numpy as np

import concourse.bacc as bacc
import concourse.tile as tile
from concourse import mybir
from concourse.bass_utils import run_bass_kernel_spmd
from concourse.masks import make_identity

F32 = mybir.dt.float32
BF16 = mybir.dt.bfloat16
F16 = mybir.dt.float16
NPBF = ml_dtypes.bfloat16
NPF16 = np.float16

B, C, H, W, HEADS = 8, 256, 64, 64, 4
N = H * W
M = N // 4
CH = C // HEADS          # 64 theta/phi channels per head
CG = (C // 2) // HEADS   # 32 g channels per head
EPS = 1e-5
NB = 128

JT = N // 512            # 8 column tiles of 512
NCHUNK = 1024
NCH = N // NCHUNK        # 4 attention i-chunks
JB = M // 128            # 8 key blocks

LN2 = float(np.log(2.0))


def _perm_headmajor(channels, heads):
    per_head = channels // heads
    r = np.arange(channels)
    return ((r % per_head) * heads + r // per_head).astype(np.int64)


def _pi_index():
    idx = np.empty(N, np.int64)
    for p in range(M):
        hp, wp = p // (W // 2), p % (W // 2)
        for d in range(4):
            dh, dw = d // 2, d % 2
            idx[p * 4 + d] = (2 * hp + dh) * W + (2 * wp + dw)
    return idx


_PERM_T = _perm_headmajor(C, HEADS)
_PERM_G = _perm_headmajor(C // 2, HEADS)
_PI = _pi_index()


def build_nc(debug_taps=False):
    Alu = mybir.AluOpType
    Act = mybir.ActivationFunctionType

    nc = bacc.Bacc("TRN2", target_bir_lowering=False)

    x_d = nc.dram_tensor("x", [C, N], F32, kind="ExternalInput")
    xb_d = nc.dram_tensor("xb", [C, N], F16, kind="ExternalInput")
    tw_d = nc.dram_tensor("tw", [C, C], F16, kind="ExternalInput")
    pw_d = nc.dram_tensor("pw", [C, C], F16, kind="ExternalInput")
    gw_d = nc.dram_tensor("gw", [C, C // 2], F16, kind="ExternalInput")
    ow_d = nc.dram_tensor("ow", [C // 2, C], BF16, kind="ExternalInput")
    gto_d = nc.dram_tensor("gto", [NB, JB], BF16, kind="ExternalInput")
    # prm columns: 0 nt_w, 1 nt_b, 2 np_w, 3 np_b, 4 c0
    prm_d = nc.dram_tensor("prm", [C, 5], F32, kind="ExternalInput")
    out_d = nc.dram_tensor("out", [C, N], F32, kind="ExternalOutput")

    with tile.TileContext(nc) as tc:
        with tc.tile_pool(name="persist", bufs=1) as pp:
            x_sb = [pp.tile([NB, N], F32, name=f"x{m}", tag=f"x{m}")
                    for m in range(2)]
            xb_sb = [pp.tile([NB, N], F16, name=f"xb{m}", tag=f"xb{m}")
                     for m in range(2)]
            tw_sb = [pp.tile([NB, C], F16, name=f"tw{k}", tag=f"tw{k}")
                     for k in range(2)]
            pw_sb = [pp.tile([NB, C], F16, name=f"pw{k}", tag=f"pw{k}")
                     for k in range(2)]
            gw_sb = [pp.tile([NB, C // 2], F16, name=f"gw{k}", tag=f"gw{k}")
                     for k in range(2)]
            ow_sb = pp.tile([NB, C], BF16, name="ow", tag="ow")
            ow3_sb = pp.tile([NB, C], BF16, name="ow3", tag="ow3")
            prm_sb = [pp.tile([NB, 5], F32, name=f"prm{m}", tag=f"prm{m}")
                      for m in range(2)]
            ident = pp.tile([NB, NB], F16, name="ident", tag="ident")
            theta_sb = [pp.tile([NB, N], F16, name=f"theta{m}", tag=f"theta{m}")
                        for m in range(2)]
            phi_sb = [pp.tile([NB, M], F16, name=f"phi{m}", tag=f"phi{m}")
                      for m in range(2)]
            g01_sb = pp.tile([2 * CG, M], F16, name="g01", tag="g01")
            g23_sb = pp.tile([2 * CG, M], F16, name="g23", tag="g23")
            gt_sb = [pp.tile([NB, JB, CG + 1], BF16, name=f"gt{h}", tag=f"gt{h}")
                     for h in range(HEADS)]
            o_sb = pp.tile([NB, N], BF16, name="ostage", tag="ostage")
            den_sb = pp.tile([NB, N], BF16, name="den", tag="den")
            lnst = pp.tile([NB, N], F32, name="lnst", tag="lnst")
            ones_c = pp.tile([NB, CG], BF16, name="ones_c", tag="ones_c")
            eps_sb = pp.tile([NB, 1], F32, name="eps", tag="eps")
            lnbias = pp.tile([NB, 1], F32, name="lnbias", tag="lnbias")
            cbias = [pp.tile([NB, 1], F32, name=f"cbias{m}", tag=f"cbias{m}")
                     for m in range(2)]

            if debug_taps:
                dbg_pt = nc.dram_tensor("dbg_pt", [NB, NCHUNK], F32,
                                        kind="ExternalOutput")
                dbg_rec = nc.dram_tensor("dbg_rec", [NB, N], F32,
                                         kind="ExternalOutput")
                dbg_o = nc.dram_tensor("dbg_o", [NB, N], F32,
                                       kind="ExternalOutput")

            nc.sync.dma_start(out=x_sb[0], in_=x_d[0:NB, :])
            nc.sync.dma_start(out=x_sb[1], in_=x_d[NB:C, :])
            nc.sync.dma_start(out=xb_sb[0], in_=xb_d[0:NB, :])
            nc.sync.dma_start(out=xb_sb[1], in_=xb_d[NB:C, :])
            for k in range(2):
                nc.sync.dma_start(out=tw_sb[k], in_=tw_d[k * NB:(k + 1) * NB, :])
                nc.sync.dma_start(out=pw_sb[k], in_=pw_d[k * NB:(k + 1) * NB, :])
                nc.sync.dma_start(out=gw_sb[k], in_=gw_d[k * NB:(k + 1) * NB, :])
            nc.sync.dma_start(out=ow_sb, in_=ow_d[:, :])
            for m in range(2):
                nc.sync.dma_start(out=prm_sb[m], in_=prm_d[m * NB:(m + 1) * NB, :])
            make_identity(nc, ident)
            nc.vector.memset(eps_sb, EPS)
            nc.vector.memset(lnbias, -40.0 * LN2)
            nc.gpsimd.memset(ones_c, 1.0)
            for h in range(HEADS):
                # denominator ones column of the o-matmul weights
                nc.sync.dma_start(out=gt_sb[h][:, :, CG:CG + 1], in_=gto_d[:, :])

            # ---- stage 1: the three convs (+ pool) --------------------
            with tc.tile_pool(name="convps", bufs=4, space="PSUM") as cps, \
                 tc.tile_pool(name="stats", bufs=1) as stp:
                t_stats = [stp.tile([NB, JT, 6], F32, name=f"ts{m}", tag=f"ts{m}")
                           for m in range(2)]
                for m in range(2):
                    for jt in range(JT):
                        ps = cps.tile([NB, 512], F32, name="conv", tag="conv")
                        js = slice(jt * 512, (jt + 1) * 512)
                        nc.tensor.matmul(ps, lhsT=tw_sb[0][:, m * NB:(m + 1) * NB],
                                         rhs=xb_sb[0][:, js], start=True, stop=False)
                        nc.tensor.matmul(ps, lhsT=tw_sb[1][:, m * NB:(m + 1) * NB],
                                         rhs=xb_sb[1][:, js], start=False, stop=True)
                        nc.vector.bn_stats(out=t_stats[m][:, jt, :], in_=ps)
                        nc.vector.tensor_copy(out=theta_sb[m][:, js], in_=ps)
                for m in range(2):
                    for jt in range(JT):
                        ps = cps.tile([NB, 512], F32, name="conv", tag="conv")
                        js = slice(jt * 512, (jt + 1) * 512)
                        nc.tensor.matmul(ps, lhsT=pw_sb[0][:, m * NB:(m + 1) * NB],
                                         rhs=xb_sb[0][:, js], start=True, stop=False)
                        nc.tensor.matmul(ps, lhsT=pw_sb[1][:, m * NB:(m + 1) * NB],
                                         rhs=xb_sb[1][:, js], start=False, stop=True)
                        nc.vector.reduce_max(
                            out=phi_sb[m][:, jt * 128:(jt + 1) * 128],
                            in_=ps.rearrange("p (a b) -> p a b", b=4),
                            axis=mybir.AxisListType.X)
                for jt in range(JT):
                    ps = cps.tile([NB, 512], F32, name="conv", tag="conv")
                    js = slice(jt * 512, (jt + 1) * 512)
                    nc.tensor.matmul(ps, lhsT=gw_sb[0], rhs=xb_sb[0][:, js],
                                     start=True, stop=False)
                    nc.tensor.matmul(ps, lhsT=gw_sb[1], rhs=xb_sb[1][:, js],
                                     start=False, stop=True)
                    for half, gdst in ((0, g01_sb), (1, g23_sb)):
                        nc.vector.reduce_max(
                            out=gdst[:, jt * 128:(jt + 1) * 128],
                            in_=ps[half * 64:(half + 1) * 64, :].rearrange(
                                "p (a b) -> p a b", b=4),
                            axis=mybir.AxisListType.X)

                # ---- stage 1b: all stats -> all Ln -> all Exp ---------
                p_stats = [stp.tile([NB, 2, 6], F32, name=f"ps{m}", tag=f"ps{m}")
                           for m in range(2)]
                g_stats = [stp.tile([2 * CG, 2, 6], F32, name=f"gs{m}",
                                    tag=f"gs{m}") for m in range(2)]
                for m in range(2):
                    for s in range(2):
                        nc.vector.bn_stats(out=p_stats[m][:, s, :],
                                           in_=phi_sb[m][:, s * 512:(s + 1) * 512])
                for half, gsrc in ((0, g01_sb), (1, g23_sb)):
                    for s in range(2):
                        nc.vector.bn_stats(out=g_stats[half][:, s, :],
                                           in_=gsrc[:, s * 512:(s + 1) * 512])
                mvs = {}
                for m in range(2):
                    mv = stp.tile([NB, 2], F32, name=f"tmv{m}", tag=f"tmv{m}")
                    nc.vector.bn_aggr(out=mv, in_=t_stats[m])
                    mvs[("t", m)] = (mv, NB)
                for m in range(2):
                    mv = stp.tile([NB, 2], F32, name=f"pmv{m}", tag=f"pmv{m}")
                    nc.vector.bn_aggr(out=mv, in_=p_stats[m])
                    mvs[("p", m)] = (mv, NB)
                for half in range(2):
                    mv = stp.tile([2 * CG, 2], F32, name=f"gmv{half}",
                                  tag=f"gmv{half}")
                    nc.vector.bn_aggr(out=mv, in_=g_stats[half])
                    mvs[("g", half)] = (mv, 2 * CG)
                # grouped Ln then grouped Exp (one table switch each)
                rstds = {}
                for key, (mv, parts) in mvs.items():
                    r = stp.tile([NB, 1], F32, name=f"rs{key[0]}{key[1]}",
                                 tag=f"rs{key[0]}{key[1]}")
                    nc.scalar.activation(out=r[0:parts, :], in_=mv[:, 1:2],
                                         func=Act.Ln, bias=eps_sb[0:parts, :],
                                         scale=1.0)
                    rstds[key] = r
                for key, (mv, parts) in mvs.items():
                    r = rstds[key]
                    nc.scalar.activation(out=r[0:parts, :], in_=r[0:parts, :],
                                         func=Act.Exp, bias=0.0, scale=-0.5)

                # ---- stage 1c: affines / folds ------------------------
                for kind, dst in (("t", theta_sb), ("p", phi_sb)):
                    for m in range(2):
                        mv, _ = mvs[(kind, m)]
                        r = rstds[(kind, m)]
                        wcol = 0 if kind == "t" else 2
                        scale = stp.tile([NB, 1], F32, name=f"sc{kind}{m}",
                                         tag=f"sc{kind}{m}")
                        nc.vector.tensor_mul(out=scale, in0=r,
                                             in1=prm_sb[m][:, wcol:wcol + 1])
                        bias = stp.tile([NB, 1], F32, name=f"bi{kind}{m}",
                                        tag=f"bi{kind}{m}")
                        nc.vector.tensor_mul(out=bias, in0=mv[:, 0:1], in1=scale)
                        nc.vector.tensor_sub(out=bias,
                                             in0=prm_sb[m][:, wcol + 1:wcol + 2],
                                             in1=bias)
                        nc.vector.tensor_scalar(out=dst[m], in0=dst[m],
                                                scalar1=scale, scalar2=bias,
                                                op0=Alu.mult, op1=Alu.add)
                alpha = stp.tile([NB, 1], F32, name="galpha", tag="galpha")
                beta = stp.tile([NB, 1], BF16, name="gbeta", tag="gbeta")
                for half in range(2):
                    mv, parts = mvs[("g", half)]
                    r = rstds[("g", half)]
                    pa = slice(half * 64, (half + 1) * 64)
                    nc.vector.tensor_copy(out=alpha[pa, :], in_=r[0:parts, :])
                    nb_t = stp.tile([2 * CG, 1], F32, name=f"gnb{half}",
                                    tag=f"gnb{half}")
                    nc.vector.tensor_mul(out=nb_t, in0=mv[:, 0:1],
                                         in1=r[0:parts, :])
                    nc.vector.tensor_scalar_mul(out=beta[pa, :], in0=nb_t,
                                                scalar1=-1.0)
                nc.vector.tensor_scalar_mul(out=ow3_sb, in0=ow_sb, scalar1=alpha)
                for m in range(2):
                    psc = cps.tile([NB, 1], F32, name="c1", tag="c1")
                    nc.tensor.matmul(psc, lhsT=ow_sb[:, m * NB:(m + 1) * NB],
                                     rhs=beta, start=True, stop=True)
                    nc.vector.tensor_add(out=cbias[m], in0=psc,
                                         in1=prm_sb[m][:, 4:5])

            # ---- stage 2: transpose g (raw) -> o-matmul weights -------
            with tc.tile_pool(name="trps", bufs=2, space="PSUM") as tps:
                for h in range(HEADS):
                    gsrc = g01_sb if h < 2 else g23_sb
                    b0 = (h % 2) * CG
                    for jb in range(JB):
                        pst = tps.tile([NB, CG], F16, name="tr", tag="tr")
                        nc.tensor.transpose(
                            pst, gsrc[b0:b0 + CG, jb * 128:(jb + 1) * 128],
                            ident[b0:b0 + CG, b0:b0 + CG])
                        nc.vector.tensor_copy(out=gt_sb[h][:, jb, 0:CG], in_=pst)

            # ---- stage 3: attention (head pairs share PE row groups) --
            with tc.tile_pool(name="stps", bufs=2, space="PSUM") as stps, \
                 tc.tile_pool(name="ops", bufs=4, space="PSUM") as ops, \
                 tc.tile_pool(name="ptp", bufs=6) as ptp:
                for hp in range(2):
                    tsrc = theta_sb[hp]
                    psrc = phi_sb[hp]
                    for c in range(NCH):
                        ps_o = [[ops.tile([CG + 1, 512], F32, name="pso",
                                          tag="pso") for _ in range(2)]
                                for _ in range(2)]  # [u][it]
                        for jb in range(JB):
                            ps_st = [stps.tile([NB, NCHUNK], F32, name="st",
                                               tag="st") for _ in range(2)]
                            for it in range(2):
                                i0 = c * NCHUNK + it * 512
                                for u in range(2):
                                    rows = slice(u * CH, (u + 1) * CH)
                                    nc.tensor.matmul(
                                        ps_st[u][:, it * 512:(it + 1) * 512],
                                        lhsT=psrc[rows, jb * 128:(jb + 1) * 128],
                                        rhs=tsrc[rows, i0:i0 + 512],
                                        start=True, stop=True)
                            pt = [ptp.tile([NB, NCHUNK], BF16, name="pt",
                                           tag="pt") for _ in range(2)]
                            for u in range(2):
                                nc.scalar.activation(out=pt[u], in_=ps_st[u],
                                                     func=Act.Exp)
                            if debug_taps and hp == 0 and c == 0 and jb == 0:
                                nc.sync.dma_start(out=dbg_pt[:, :], in_=pt[0])
                            for it in range(2):
                                for u in range(2):
                                    h = 2 * hp + u
                                    nc.tensor.matmul(
                                        ps_o[u][it], lhsT=gt_sb[h][:, jb, :],
                                        rhs=pt[u][:, it * 512:(it + 1) * 512],
                                        start=(jb == 0), stop=(jb == JB - 1))
                        for it in range(2):
                            i0 = c * NCHUNK + it * 512
                            for u in range(2):
                                h = 2 * hp + u
                                nc.vector.tensor_copy(
                                    out=o_sb[h * CG:(h + 1) * CG, i0:i0 + 512],
                                    in_=ps_o[u][it][0:CG, :])
                                nc.vector.tensor_copy(
                                    out=den_sb[h * CG:h * CG + 1, i0:i0 + 512],
                                    in_=ps_o[u][it][CG:CG + 1, :])

            # ---- stage 4: o /= softmax denominator --------------------
            with tc.tile_pool(name="rbps", bufs=2, space="PSUM") as rbps, \
                 tc.tile_pool(name="rbp2", bufs=2) as rbp2, \
                 tc.tile_pool(name="ocps", bufs=2, space="PSUM") as ocps:
                if debug_taps:
                    nc.sync.dma_start(out=dbg_o[:, :], in_=o_sb)
                # broadcast raw denominators to all partitions via K=1
                # ones-matmuls, then recip = exp(-ln(d*2^-40) - 40ln2)
                # (ln's table only covers |x| <= 2^64; denominators ~6e24).
                for jt in range(JT):
                    js = slice(jt * 512, (jt + 1) * 512)
                    psr = rbps.tile([NB, 512], F32, name="rb", tag="rb")
                    for h in range(HEADS):
                        nc.tensor.matmul(psr[h * CG:(h + 1) * CG, :],
                                         lhsT=ones_c[h * CG:h * CG + 1, :],
                                         rhs=den_sb[h * CG:h * CG + 1, js],
                                         start=True, stop=True,
                                         tile_position=(h * CG, h * CG))
                    nc.scalar.activation(out=lnst[:, js], in_=psr, func=Act.Ln,
                                         scale=2.0 ** -40)
                for jt in range(JT):
                    js = slice(jt * 512, (jt + 1) * 512)
                    recb = rbp2.tile([NB, 512], F32, name="recb", tag="recb")
                    nc.scalar.activation(out=recb, in_=lnst[:, js],
                                         func=Act.Exp, scale=-1.0, bias=lnbias)
                    nc.vector.tensor_mul(out=o_sb[:, js], in0=o_sb[:, js],
                                         in1=recb)
                if debug_taps:
                    nc.sync.dma_start(out=dbg_rec[:, :], in_=lnst)

                # ---- stage 5: output conv + residual ------------------
                for m in range(2):
                    for jt in range(JT):
                        js = slice(jt * 512, (jt + 1) * 512)
                        pso = ocps.tile([NB, 512], F32, name="oc", tag="oc")
                        nc.tensor.matmul(pso, lhsT=ow3_sb[:, m * NB:(m + 1) * NB],
                                         rhs=o_sb[:, js], start=True, stop=True)
                        # out = (oc + cbias) + x, written in place over x
                        nc.vector.scalar_tensor_tensor(
                            out=x_sb[m][:, js], in0=pso, scalar=cbias[m],
                            in1=x_sb[m][:, js], op0=Alu.add, op1=Alu.add)
                        nc.sync.dma_start(out=out_d[m * NB:(m + 1) * NB, js],
                                          in_=x_sb[m][:, js])

    nc.compile()
    return nc


def prepare_core_inputs(inputs):
    x = np.asarray(inputs["x"], np.float32)
    theta_w = np.asarray(inputs["theta_w"], np.float32)
    phi_w = np.asarray(inputs["phi_w"], np.float32)
    g_w = np.asarray(inputs["g_w"], np.float32)
    o_w = np.asarray(inputs["o_w"], np.float32)
    nt_w = np.asarray(inputs["nt_w"], np.float32)
    nt_b = np.asarray(inputs["nt_b"], np.float32)
    np_w = np.asarray(inputs["np_w"], np.float32)
    np_b = np.asarray(inputs["np_b"], np.float32)
    ng_w = np.asarray(inputs["ng_w"], np.float32)
    ng_b = np.asarray(inputs["ng_b"], np.float32)
    gamma = float(np.asarray(inputs["gamma"]))

    xf = np.ascontiguousarray(x.reshape(B, C, N)[:, :, _PI])
    tw = np.ascontiguousarray(theta_w[_PERM_T, :].T)
    pw = np.ascontiguousarray(phi_w[_PERM_T, :].T)
    gw = np.ascontiguousarray(g_w[_PERM_G, :].T)
    ow_p = o_w[:, _PERM_G]
    ow2 = ow_p * (gamma * ng_w[_PERM_G])[None, :]
    owT = np.ascontiguousarray(ow2.T)
    c0 = gamma * (ow_p @ ng_b[_PERM_G])
    prm = np.zeros((C, 5), np.float32)
    prm[:, 0] = nt_w[_PERM_T]
    prm[:, 1] = nt_b[_PERM_T]
    prm[:, 2] = np_w[_PERM_T]
    prm[:, 3] = np_b[_PERM_T]
    prm[:, 4] = c0
    shared = {
        "tw": tw.astype(NPF16), "pw": pw.astype(NPF16), "gw": gw.astype(NPF16),
        "ow": owT.astype(NPBF), "gto": np.ones((NB, JB), NPBF),
        "prm": prm,
    }
    return [dict(shared, x=xf[b], xb=xf[b].astype(NPF16)) for b in range(B)]


def assemble_output(outs):
    full = np.stack(outs, 0)
    res = np.empty((B, C, N), np.float32)
    res[:, :, _PI] = full
    return res.reshape(B, C, H, W)


_NC_CACHE = []


def _get_nc():
    if not _NC_CACHE:
        _NC_CACHE.append(build_nc())
    return _NC_CACHE[0]


def run_on_hw(inputs, trace=False):
    nc = _get_nc()
    in_maps = prepare_core_inputs(inputs)
    res = run_bass_kernel_spmd(nc, in_maps, core_ids=list(range(B)), trace=trace)
    out = assemble_output([res.results[i]["out"] for i in range(B)])
    return out, res


def kernel(**inputs):
    out, _ = run_on_hw(inputs, trace=False)
    return out
